# revision 14
# baseline (speedup 1.0000x reference)
"""GAT layer (nn_GATLayer) on 8 Trainium2 NeuronCores — gather-free design, v2.

Sharding: edges + output nodes sharded by dst node via balanced graph
partitioning (edge-cut per the hint); all FP compute on device; host does
integer graph partitioning and layout/weight reshuffling only (no FP math on
x beyond dtype conversion).

v2 changes vs v1 (282us):
  - PSUM pending-zero semantics exploited: start=True marks the whole 2KB
    bank pending-zero; later start=False writes to still-pending bytes
    OVERWRITE, writes to already-written bytes accumulate.  So per chunk
    only: fe0 (136c, start=T), fe1 (136c, start=F overwrite), er (16c
    strided-out accumulate), agg (272c).  The padded 272c claims / 136c er
    scatters / zcat zero matmul of v1 are gone: 816 -> 560 moving cols,
    7 -> 4 matmuls (= LDWEIGHTS) per chunk.
  - s-matmul folded into the agg matmul: the msg tile holds
    [msg_b0|msg_b1|ex_b0|ex_b1] (272 cols) so one matmul accumulates both
    agg and the softmax denominator.
  - w >= 0 so Lrelu(z*w) = w*Lrelu(z): per-chunk Prelu(scale=w) becomes a
    per-pair Prelu (no scale) + tiny gpsimd w-multiply + per-pair Exp.
    pf tiles are 2-bank PSUM tiles holding a chunk pair, so Prelu and the
    msg multiply batch across the pair.
  - ex broadcast over dh is materialized by an on-chip stride-0-read DMA,
    making the msg multiply a contiguous DVE op per pair.
  - agg matmuls trail the fe stream by LAG pairs (software pipelining) so
    the PE never stalls on the ACT->gpsimd->ACT->DMA->DVE msg chain.
  - Epilogue transpose via dma_start_transpose (XBAR): kills 40 PE
    transposes + 40 ACT copies + a PSUM bank.
  - S/ST shipped as fp8e4 (0/1 exact; matmul allows fp8 lhsT x bf16 rhs):
    halves their HBM traffic.  Output written bf16 (host casts to f32).
  - Host graph partition balances edges over the 160 (core, group) bins
    (each bin <= 128 dst nodes) so every group needs the same minimal
    chunk count (TC 350 -> 320); output rows un-permuted on host (integer
    gather only).
"""

import numpy as np
import ml_dtypes
import heapq
from collections import deque
from contextlib import ExitStack

import concourse.bass as bass
import concourse.bacc as bacc
import concourse.tile as tile
from concourse import mybir
from concourse.bass_utils import run_bass_kernel_spmd

B, N, D, H, DH, OUT = 2, 20000, 128, 8, 16, 64
E = 320000
NEG_SLOPE = 0.1
NCORES = 8
NG = 20                      # groups of <=128 dst nodes per core
NPC = NG * 128               # padded own-dst rows per core (2560)
F32 = mybir.dt.float32
BF16 = mybir.dt.bfloat16
FP8 = mybir.dt.float8e4
MULT = mybir.AluOpType.mult
ADD = mybir.AluOpType.add
SLAB = 16                    # chunks per DMA slab
LAG = 5                      # pairs the agg matmuls trail behind the fe stream

LAST_RESULTS = None  # test harness can inspect exec_time_ns / profile


def _ap(t, off, dims):
    return bass.AP(tensor=t.tensor, offset=t.offset + off, ap=[t.ap[0]] + dims)


def _build_program(NC, bias_nonzero):
    TC = sum(NC)
    nc = bacc.Bacc(
        "TRN2", target_bir_lowering=False, debug=False, num_devices=NCORES
    )
    xeT0_d = nc.dram_tensor("xeT0", [128, TC * 128], BF16, kind="ExternalInput").ap()
    xeT1_d = nc.dram_tensor("xeT1", [128, TC * 128], BF16, kind="ExternalInput").ap()
    s_d = nc.dram_tensor("smat", [128, TC * 128], FP8, kind="ExternalInput").ap()
    st_d = nc.dram_tensor("stmat", [128, TC * 128], FP8, kind="ExternalInput").ap()
    wcol_d = nc.dram_tensor("wcol", [128, TC], F32, kind="ExternalInput").ap()
    xgT_d = nc.dram_tensor("xgT", [128, 2 * NPC], BF16, kind="ExternalInput").ap()
    wcat_d = nc.dram_tensor("wcat", [128, 136], BF16, kind="ExternalInput").ap()
    wer_d = nc.dram_tensor("wer", [128, 8], BF16, kind="ExternalInput").ap()
    wblk_d = nc.dram_tensor("wblk", [128, 512], BF16, kind="ExternalInput").ap()
    bout_d = nc.dram_tensor("bout", [128, 512], F32, kind="ExternalInput").ap()
    cel_d = nc.dram_tensor("cel", [128, 8], F32, kind="ExternalInput").ap()
    out_d = nc.dram_tensor("out", [B, NPC, 512], BF16, kind="ExternalOutput").ap()

    cbase = [0] + [int(v) for v in np.cumsum(NC)]

    with ExitStack() as ctx:
        tc = ctx.enter_context(tile.TileContext(nc))
        singles = ctx.enter_context(tc.tile_pool(name="singles", bufs=1))
        wcat_sb = singles.tile([128, 136], BF16)
        nc.sync.dma_start(wcat_sb, wcat_d)
        wer_sb = singles.tile([128, 8], BF16)
        nc.sync.dma_start(wer_sb, wer_d)
        wblk_sb = singles.tile([128, 512], BF16)
        nc.sync.dma_start(wblk_sb, wblk_d)
        bout_sb = singles.tile([128, 512], F32)
        nc.sync.dma_start(bout_sb, bout_d)
        cel_sb = singles.tile([128, 8], F32)
        nc.sync.dma_start(cel_sb, cel_d)
        wcol_sb = singles.tile([128, TC], F32)
        nc.sync.dma_start(wcol_sb, wcol_d)
        xg_sb = singles.tile([128, 2 * NPC], BF16)
        nc.sync.dma_start(xg_sb, xgT_d)
        # erp[m, g*16 + b*8 + h] = er of own-dst node (g, m) + (bel+ber)[h]
        erp = singles.tile([128, NG * 16], BF16)

        # ---- er pass ---------------------------------------------------------
        with ExitStack() as erctx:
            pp_er = erctx.enter_context(
                tc.tile_pool(name="pp_er", bufs=2, space="PSUM")
            )
            for b in range(2):
                for g in range(NG):
                    ps = pp_er.tile([128, 8], F32, tag="er")
                    nc.tensor.matmul(
                        ps, xg_sb[:, b * NPC + g * 128 : b * NPC + (g + 1) * 128],
                        wer_sb, start=True, stop=True,
                    )
                    nc.vector.tensor_tensor(
                        erp[:, g * 16 + b * 8 : g * 16 + b * 8 + 8],
                        ps, cel_sb, ADD,
                    )

        # ---- pools -----------------------------------------------------------
        xs0 = ctx.enter_context(tc.tile_pool(name="xs0", bufs=3))
        xs1 = ctx.enter_context(tc.tile_pool(name="xs1", bufs=3))
        ssl = ctx.enter_context(tc.tile_pool(name="ssl", bufs=3))
        stl = ctx.enter_context(tc.tile_pool(name="stl", bufs=3))
        p_u = ctx.enter_context(tc.tile_pool(name="p_u", bufs=4))
        p_msg = ctx.enter_context(tc.tile_pool(name="p_msg", bufs=2 * LAG + 4))
        p_fin = ctx.enter_context(tc.tile_pool(name="p_fin", bufs=6))
        pp_q = ctx.enter_context(tc.tile_pool(name="pp_q", bufs=3, space="PSUM"))
        pp_agg = ctx.enter_context(tc.tile_pool(name="pp_agg", bufs=1, space="PSUM"))
        pp_epi = ctx.enter_context(tc.tile_pool(name="pp_epi", bufs=1, space="PSUM"))

        slabs = {}

        def slab_tiles(cg):
            si = cg // SLAB
            if si not in slabs:
                n = min(SLAB, TC - si * SLAB) * 128
                o = si * SLAB * 128
                x0 = xs0.tile([128, n], BF16, tag="x0")
                nc.sync.dma_start(x0, xeT0_d[:, o : o + n])
                x1 = xs1.tile([128, n], BF16, tag="x1")
                nc.sync.dma_start(x1, xeT1_d[:, o : o + n])
                sm = ssl.tile([128, n], FP8, tag="sm")
                nc.sync.dma_start(sm, s_d[:, o : o + n])
                st = stl.tile([128, n], FP8, tag="st")
                nc.sync.dma_start(st, st_d[:, o : o + n])
                slabs[si] = (x0, x1, sm, st)
            k = (cg % SLAB) * 128
            x0, x1, sm, st = slabs[si]
            return (
                x0[:, k : k + 128], x1[:, k : k + 128],
                sm[:, k : k + 128], st[:, k : k + 128],
            )

        aggs = {}  # g -> agg_ps tile (allocated by the group's first agg)

        def emit_pair(g, c0):
            """fe matmuls + u/ex/msg chain for chunks [c0, c0+np_) of group g.
            Returns a closure that emits the (lagged) agg matmuls."""
            ncg = NC[g]
            np_ = min(2, ncg - c0)
            pft = pp_q.tile([128, 1024], F32, tag="pf")
            sts = []
            for i in range(np_):
                cg = cbase[g] + c0 + i
                xe0, xe1, s_t, st_t = slab_tiles(cg)
                sts.append(s_t)
                pf = pft[:, i * 512 : i * 512 + 272]
                nc.tensor.matmul(pf[:, 0:136], xe0, wcat_sb,
                                 start=True, stop=False)
                nc.tensor.matmul(pf[:, 136:272], xe1, wcat_sb,
                                 start=False, stop=False)
                # el += er (strided out: cols 128:136 and 264:272)
                nc.tensor.matmul(
                    _ap(pf, 128, [[136, 2], [1, 8]]),
                    st_t, erp[:, g * 16 : g * 16 + 16],
                    start=False, stop=True,
                )
            u_p = p_u.tile([128, np_ * 16], F32, tag="u")
            msg_p = p_msg.tile([128, np_ * 272], BF16, tag="msg")
            # leaky-relu for the pair (w folded in below; w >= 0)
            nc.scalar.activation(
                _ap(u_p, 0, [[16, np_], [8, 2], [1, 8]]),
                _ap(pft, 128, [[512, np_], [136, 2], [1, 8]]),
                mybir.ActivationFunctionType.Prelu,
                alpha=NEG_SLOPE,
            )
            # u *= w  (broadcast w over the 16 (b,h) slots)
            nc.gpsimd.tensor_tensor(
                u_p,
                u_p,
                _ap(wcol_sb, cbase[g] + c0, [[1, np_], [0, 16]]),
                MULT,
            )
            # ex = exp(u) -> msg_p[i*272 + 256 + b*8 + h]
            nc.scalar.activation(
                _ap(msg_p, 256, [[272, np_], [8, 2], [1, 8]]),
                _ap(u_p, 0, [[16, np_], [8, 2], [1, 8]]),
                mybir.ActivationFunctionType.Exp,
            )
            # broadcast-expand ex over dh into msg_p[i*272 + b*128 + ...]
            # feat columns are dh-major (j*8+h) so the 8-head block replicates
            # as a contiguous unit (DMA fastest dim must be stride-contiguous)
            for i in range(np_):
                for b in range(2):
                    nc.sync.dma_start(
                        _ap(msg_p, i * 272 + b * 128, [[8, 16], [1, 8]]),
                        _ap(msg_p, i * 272 + 256 + b * 8, [[0, 16], [1, 8]]),
                    )
            # msg = feat * ex  (in place, on the pre-expanded ex)
            nc.vector.tensor_tensor(
                _ap(msg_p, 0, [[272, np_], [128, 2], [1, 128]]),
                _ap(pft, 0, [[512, np_], [136, 2], [1, 128]]),
                _ap(msg_p, 0, [[272, np_], [128, 2], [1, 128]]),
                MULT,
            )

            def agg_closure():
                ncg_ = NC[g]
                if c0 == 0:
                    agg_t = pp_agg.tile([128, 512], F32, tag="agg")
                    aggs[g] = agg_t
                agg_ps = aggs[g]
                for i in range(np_):
                    c = c0 + i
                    nc.tensor.matmul(
                        agg_ps[:, 0:272], sts[i],
                        msg_p[:, i * 272 : (i + 1) * 272],
                        start=(c == 0), stop=(c == ncg_ - 1),
                    )

            return agg_closure

        def emit_epilogue(g):
            def closure():
                agg_ps = aggs.pop(g)
                sinv = p_fin.tile([128, 16], F32, tag="sinv")
                nc.vector.tensor_scalar_add(sinv, agg_ps[:, 256:272], 1e-30)
                nc.vector.reciprocal(sinv, sinv)
                aggn = p_fin.tile([128, 256], BF16, tag="aggn")
                # agg columns are (b, dh, h): sinv (b,h) broadcasts over dh
                nc.vector.tensor_tensor(
                    _ap(aggn, 0, [[128, 2], [8, 16], [1, 8]]),
                    _ap(agg_ps, 0, [[128, 2], [8, 16], [1, 8]]),
                    _ap(sinv, 0, [[8, 2], [0, 16], [1, 8]]),
                    MULT,
                )
                for b in range(2):
                    aggnT = p_fin.tile([128, 128], BF16, tag="aggnT")
                    nc.sync.dma_start_transpose(
                        aggnT, aggn[:, b * 128 : (b + 1) * 128]
                    )
                    psr = pp_epi.tile([128, 512], F32, tag="psr")
                    nc.tensor.matmul(psr, aggnT, wblk_sb, start=True, stop=True)
                    rst = p_fin.tile([128, 512], BF16, tag="rst")
                    if bias_nonzero:
                        nc.vector.tensor_tensor(rst, psr, bout_sb, ADD)
                    elif b == 0:
                        nc.scalar.activation(
                            rst, psr, mybir.ActivationFunctionType.Copy
                        )
                    else:
                        nc.vector.tensor_copy(rst, psr)
                    nc.sync.dma_start(out_d[b, g * 128 : (g + 1) * 128, :], rst)
            return closure

        pend = deque()
        for g in range(NG):
            for c0 in range(0, NC[g], 2):
                pend.append(emit_pair(g, c0))
                while len(pend) > LAG:
                    pend.popleft()()
            pend.append(emit_epilogue(g))
        while pend:
            pend.popleft()()

    nc.finalize()
    return nc


def _prep_host(x, src, dst, w, W_fc, b_fc, attn_l, attn_r, W_out, b_out):
    bf = ml_dtypes.bfloat16
    f8 = ml_dtypes.float8_e4m3
    x = np.asarray(x, np.float32)
    src = np.asarray(src).astype(np.int64)
    dst = np.asarray(dst).astype(np.int64)
    w = np.asarray(w, np.float32)
    W_fc = np.asarray(W_fc, np.float32)
    b_fc = np.asarray(b_fc, np.float32)
    al = np.asarray(attn_l, np.float32).reshape(H, DH)
    ar = np.asarray(attn_r, np.float32).reshape(H, DH)
    W_out = np.asarray(W_out, np.float32)
    b_out = np.asarray(b_out, np.float32)

    WfcT = np.ascontiguousarray(W_fc.T)                       # (d_in, d_out)
    W_el = np.einsum("dhk,hk->dh", WfcT.reshape(D, H, DH), al)
    # feat columns in dh-major order: col j*8+h = original dim h*16+j
    dperm = np.array([h * DH + j for j in range(DH) for h in range(H)])
    wcat = np.concatenate([WfcT[:, dperm], W_el], axis=1).astype(bf)  # (128,136)
    wer = np.einsum("dhk,hk->dh", WfcT.reshape(D, H, DH), ar).astype(bf)
    bel = np.einsum("hk,hk->h", b_fc.reshape(H, DH), al)
    ber = np.einsum("hk,hk->h", b_fc.reshape(H, DH), ar)
    cel = np.tile((bel + ber).astype(np.float32), (128, 1))   # (128, 8)
    wblk = np.zeros((D, 512), np.float32)
    for h in range(H):
        wblk[h * DH : (h + 1) * DH, h * OUT : (h + 1) * OUT] = W_out.T
    bfc_blk = b_fc @ wblk                                     # (512,)
    wblk = wblk[dperm, :]  # rows follow the dh-major feat layout
    bout_eff = np.tile(np.tile(b_out, H) + bfc_blk, (128, 1)).astype(np.float32)
    bias_nonzero = bool(np.abs(bout_eff).max() > 0)
    wblk = wblk.astype(bf)

    # ---- balanced graph partition: 160 (core, group) bins, <=128 nodes ----
    deg = np.bincount(dst, minlength=N)
    NB = NCORES * NG
    order_nodes = np.argsort(-deg, kind="stable")
    heap = [(0, 0, bid) for bid in range(NB)]  # (edge_total, node_cnt, bid)
    heapq.heapify(heap)
    node_bin = np.empty(N, np.int64)
    node_lid = np.empty(N, np.int64)
    for n in order_nodes:
        tot, cnt, bid = heapq.heappop(heap)
        node_bin[n] = bid
        node_lid[n] = cnt
        cnt += 1
        tot += int(deg[n])
        if cnt < 128:
            heapq.heappush(heap, (tot, cnt, bid))
    # bid = k * NG + g
    bin_k = node_bin // NG
    bin_g = node_bin % NG

    order = np.argsort(node_bin[dst], kind="stable")
    dsts, srcs, ws = dst[order], src[order], w[order]
    bins_sorted = node_bin[dsts]
    bounds = np.searchsorted(bins_sorted, np.arange(NB + 1))
    cnts = (bounds[1:] - bounds[:-1]).reshape(NCORES, NG)
    NC = np.maximum(1, ((cnts + 127) // 128).max(axis=0)).astype(int)  # (NG,)
    cbase = np.concatenate([[0], np.cumsum(NC)]).astype(int)
    TC = int(cbase[-1])

    xT = [np.ascontiguousarray(x[b].T.astype(bf)) for b in range(B)]  # (128, N)

    xeT = np.zeros((2, NCORES, 128, TC * 128), bf)
    S = np.zeros((NCORES, 128, TC * 128), f8)
    ST = np.zeros((NCORES, 128, TC * 128), f8)
    wcol = np.zeros((NCORES, 128, TC), np.float32)
    xgT = np.zeros((NCORES, 128, 2 * NPC), bf)
    # node -> (k, g, lid); own-dst columns for the er pass + out un-permute
    perm_node = np.full((NCORES, NPC), -1, np.int64)
    rows = bin_g * 128 + node_lid
    for k in range(NCORES):
        m = bin_k == np.int64(k)
        perm_node[k, rows[m]] = np.nonzero(m)[0]
        for b in range(B):
            xgT[k, :, b * NPC + rows[m]] = xT[b][:, m].T
    for bid in range(NB):
        k, g = bid // NG, bid % NG
        i0, i1 = bounds[bid], bounds[bid + 1]
        cnt = int(i1 - i0)
        if cnt == 0:
            continue
        j = np.arange(cnt)
        ch = cbase[g] + j // 128                           # global chunk
        sl = j % 128                                       # slot (edge row)
        lid = node_lid[dsts[i0:i1]]
        cols = ch * 128
        S[k, sl, cols + lid] = 1
        ST[k, lid, cols + sl] = 1
        wcol[k, sl, ch] = ws[i0:i1]
        for b in range(B):
            xeT[b, k, :, cols + sl] = xT[b][:, srcs[i0:i1]].T
    return (
        xeT, S, ST, wcol, xgT, wcat, wer, wblk, bout_eff, cel,
        list(map(int, NC)), bias_nonzero, perm_node,
    )


def kernel(vt=None, x=None, src=None, dst=None, w=None, W_fc=None, b_fc=None,
           attn_l=None, attn_r=None, W_out=None, b_out=None, **_ignored):
    global LAST_RESULTS
    (xeT, S, ST, wcol, xgT, wcat, wer, wblk, bout_eff, cel,
     NC, bias_nonzero, perm_node) = _prep_host(
        x, src, dst, w, W_fc, b_fc, attn_l, attn_r, W_out, b_out
    )
    nc = _build_program(NC, bias_nonzero)
    in_maps = []
    for k in range(NCORES):
        in_maps.append(
            dict(
                xeT0=np.ascontiguousarray(xeT[0, k]),
                xeT1=np.ascontiguousarray(xeT[1, k]),
                smat=np.ascontiguousarray(S[k]),
                stmat=np.ascontiguousarray(ST[k]),
                wcol=np.ascontiguousarray(wcol[k]),
                xgT=np.ascontiguousarray(xgT[k]),
                wcat=wcat,
                wer=wer,
                wblk=wblk,
                bout=bout_eff,
                cel=cel,
            )
        )
    res = run_bass_kernel_spmd(nc, in_maps, core_ids=list(range(NCORES)))
    LAST_RESULTS = res
    outs = [np.asarray(res.results[k]["out"], np.float32) for k in range(NCORES)]
    dev = np.concatenate(outs, axis=1)                   # (B, NCORES*NPC, 512)
    nodes = np.concatenate([perm_node[k] for k in range(NCORES)])
    valid = nodes >= 0
    full = np.empty((B, N, 512), np.float32)
    full[:, nodes[valid]] = dev[:, valid]
    return np.ascontiguousarray(full.reshape(B, N, H, OUT))


# revision 15
# speedup vs baseline: 3.6198x; 3.6198x over previous
"""GAT layer (nn_GATLayer) on 8 Trainium2 NeuronCores — gather-free design, v2.

Sharding: edges + output nodes sharded by dst node via balanced graph
partitioning (edge-cut per the hint); all FP compute on device; host does
integer graph partitioning and layout/weight reshuffling only (no FP math on
x beyond dtype conversion).

v2 changes vs v1 (282us):
  - PSUM pending-zero semantics exploited: start=True marks the whole 2KB
    bank pending-zero; later start=False writes to still-pending bytes
    OVERWRITE, writes to already-written bytes accumulate.  So per chunk
    only: fe0 (136c, start=T), fe1 (136c, start=F overwrite), er (16c
    strided-out accumulate), agg (272c).  The padded 272c claims / 136c er
    scatters / zcat zero matmul of v1 are gone: 816 -> 560 moving cols,
    7 -> 4 matmuls (= LDWEIGHTS) per chunk.
  - s-matmul folded into the agg matmul: the msg tile holds
    [msg_b0|msg_b1|ex_b0|ex_b1] (272 cols) so one matmul accumulates both
    agg and the softmax denominator.
  - w >= 0 so Lrelu(z*w) = w*Lrelu(z): per-chunk Prelu(scale=w) becomes a
    per-pair Prelu (no scale) + tiny gpsimd w-multiply + per-pair Exp.
    pf tiles are 2-bank PSUM tiles holding a chunk pair, so Prelu and the
    msg multiply batch across the pair.
  - ex broadcast over dh is materialized by an on-chip stride-0-read DMA,
    making the msg multiply a contiguous DVE op per pair.
  - agg matmuls trail the fe stream by LAG pairs (software pipelining) so
    the PE never stalls on the ACT->gpsimd->ACT->DMA->DVE msg chain.
  - Epilogue transpose via dma_start_transpose (XBAR): kills 40 PE
    transposes + 40 ACT copies + a PSUM bank.
  - S/ST shipped as fp8e4 (0/1 exact; matmul allows fp8 lhsT x bf16 rhs):
    halves their HBM traffic.  Output written bf16 (host casts to f32).
  - Host graph partition balances edges over the 160 (core, group) bins
    (each bin <= 128 dst nodes) so every group needs the same minimal
    chunk count (TC 350 -> 320); output rows un-permuted on host (integer
    gather only).
"""

import numpy as np
import ml_dtypes
import heapq
from collections import deque
from contextlib import ExitStack

import concourse.bass as bass
import concourse.bacc as bacc
import concourse.tile as tile
from concourse import mybir
from concourse.bass_utils import run_bass_kernel_spmd

B, N, D, H, DH, OUT = 2, 20000, 128, 8, 16, 64
E = 320000
NEG_SLOPE = 0.1
NCORES = 8
NG = 20                      # groups of <=128 dst nodes per core
NPC = NG * 128               # padded own-dst rows per core (2560)
F32 = mybir.dt.float32
BF16 = mybir.dt.bfloat16
FP8 = mybir.dt.float8e4
MULT = mybir.AluOpType.mult
ADD = mybir.AluOpType.add
SLAB = 16                    # chunks per DMA slab
LAG = 5                      # pairs the agg matmuls trail behind the fe stream

LAST_RESULTS = None  # test harness can inspect exec_time_ns / profile


def _ap(t, off, dims):
    return bass.AP(tensor=t.tensor, offset=t.offset + off, ap=[t.ap[0]] + dims)


def _build_program(NC, bias_nonzero):
    TC = sum(NC)
    nc = bacc.Bacc(
        "TRN2", target_bir_lowering=False, debug=False, num_devices=NCORES
    )
    xeT0_d = nc.dram_tensor("xeT0", [128, TC * 128], BF16, kind="ExternalInput").ap()
    xeT1_d = nc.dram_tensor("xeT1", [128, TC * 128], BF16, kind="ExternalInput").ap()
    s_d = nc.dram_tensor("smat", [128, TC * 128], FP8, kind="ExternalInput").ap()
    st_d = nc.dram_tensor("stmat", [128, TC * 128], FP8, kind="ExternalInput").ap()
    wcol_d = nc.dram_tensor("wcol", [128, TC], F32, kind="ExternalInput").ap()
    xgT_d = nc.dram_tensor("xgT", [128, 2 * NPC], BF16, kind="ExternalInput").ap()
    wcat_d = nc.dram_tensor("wcat", [128, 136], BF16, kind="ExternalInput").ap()
    wer_d = nc.dram_tensor("wer", [128, 8], BF16, kind="ExternalInput").ap()
    wblk_d = nc.dram_tensor("wblk", [128, 512], BF16, kind="ExternalInput").ap()
    bout_d = nc.dram_tensor("bout", [128, 512], F32, kind="ExternalInput").ap()
    cel_d = nc.dram_tensor("cel", [128, 8], F32, kind="ExternalInput").ap()
    out_d = nc.dram_tensor("out", [B, NPC, 512], BF16, kind="ExternalOutput").ap()

    cbase = [0] + [int(v) for v in np.cumsum(NC)]

    with ExitStack() as ctx:
        tc = ctx.enter_context(tile.TileContext(nc))
        singles = ctx.enter_context(tc.tile_pool(name="singles", bufs=1))
        wcat_sb = singles.tile([128, 136], BF16)
        nc.sync.dma_start(wcat_sb, wcat_d)
        wer_sb = singles.tile([128, 8], BF16)
        nc.sync.dma_start(wer_sb, wer_d)
        wblk_sb = singles.tile([128, 512], BF16)
        nc.sync.dma_start(wblk_sb, wblk_d)
        bout_sb = singles.tile([128, 512], F32)
        nc.sync.dma_start(bout_sb, bout_d)
        cel_sb = singles.tile([128, 8], F32)
        nc.sync.dma_start(cel_sb, cel_d)
        wcol_sb = singles.tile([128, TC], F32)
        nc.sync.dma_start(wcol_sb, wcol_d)
        xg_sb = singles.tile([128, 2 * NPC], BF16)
        nc.sync.dma_start(xg_sb, xgT_d)
        # erp[m, g*16 + b*8 + h] = er of own-dst node (g, m) + (bel+ber)[h]
        erp = singles.tile([128, NG * 16], BF16)

        # ---- er pass ---------------------------------------------------------
        with ExitStack() as erctx:
            pp_er = erctx.enter_context(
                tc.tile_pool(name="pp_er", bufs=2, space="PSUM")
            )
            for b in range(2):
                for g in range(NG):
                    ps = pp_er.tile([128, 8], F32, tag="er")
                    nc.tensor.matmul(
                        ps, xg_sb[:, b * NPC + g * 128 : b * NPC + (g + 1) * 128],
                        wer_sb, start=True, stop=True,
                    )
                    nc.vector.tensor_tensor(
                        erp[:, g * 16 + b * 8 : g * 16 + b * 8 + 8],
                        ps, cel_sb, ADD,
                    )

        # ---- pools -----------------------------------------------------------
        xs0 = ctx.enter_context(tc.tile_pool(name="xs0", bufs=3))
        xs1 = ctx.enter_context(tc.tile_pool(name="xs1", bufs=3))
        ssl = ctx.enter_context(tc.tile_pool(name="ssl", bufs=3))
        stl = ctx.enter_context(tc.tile_pool(name="stl", bufs=3))
        p_u = ctx.enter_context(tc.tile_pool(name="p_u", bufs=4))
        p_msg = ctx.enter_context(tc.tile_pool(name="p_msg", bufs=2 * LAG + 4))
        p_fin = ctx.enter_context(tc.tile_pool(name="p_fin", bufs=6))
        pp_q = ctx.enter_context(tc.tile_pool(name="pp_q", bufs=3, space="PSUM"))
        pp_agg = ctx.enter_context(tc.tile_pool(name="pp_agg", bufs=1, space="PSUM"))
        pp_epi = ctx.enter_context(tc.tile_pool(name="pp_epi", bufs=1, space="PSUM"))

        slabs = {}

        def slab_tiles(cg):
            si = cg // SLAB
            if si not in slabs:
                n = min(SLAB, TC - si * SLAB) * 128
                o = si * SLAB * 128
                x0 = xs0.tile([128, n], BF16, tag="x0")
                nc.sync.dma_start(x0, xeT0_d[:, o : o + n])
                x1 = xs1.tile([128, n], BF16, tag="x1")
                nc.sync.dma_start(x1, xeT1_d[:, o : o + n])
                sm = ssl.tile([128, n], FP8, tag="sm")
                nc.sync.dma_start(sm, s_d[:, o : o + n])
                st = stl.tile([128, n], FP8, tag="st")
                nc.sync.dma_start(st, st_d[:, o : o + n])
                slabs[si] = (x0, x1, sm, st)
            k = (cg % SLAB) * 128
            x0, x1, sm, st = slabs[si]
            return (
                x0[:, k : k + 128], x1[:, k : k + 128],
                sm[:, k : k + 128], st[:, k : k + 128],
            )

        aggs = {}  # g -> agg_ps tile (allocated by the group's first agg)

        def emit_pair(g, c0):
            """fe matmuls + u/ex/msg chain for chunks [c0, c0+np_) of group g.
            Returns a closure that emits the (lagged) agg matmuls."""
            ncg = NC[g]
            np_ = min(2, ncg - c0)
            pft = pp_q.tile([128, 1024], F32, tag="pf")
            sts = []
            for i in range(np_):
                cg = cbase[g] + c0 + i
                xe0, xe1, s_t, st_t = slab_tiles(cg)
                sts.append(s_t)
                pf = pft[:, i * 512 : i * 512 + 272]
                nc.tensor.matmul(pf[:, 0:136], xe0, wcat_sb,
                                 start=True, stop=False)
                nc.tensor.matmul(pf[:, 136:272], xe1, wcat_sb,
                                 start=False, stop=False)
                # el += er (strided out: cols 128:136 and 264:272)
                nc.tensor.matmul(
                    _ap(pf, 128, [[136, 2], [1, 8]]),
                    st_t, erp[:, g * 16 : g * 16 + 16],
                    start=False, stop=True,
                )
            u_p = p_u.tile([128, np_ * 16], F32, tag="u")
            msg_p = p_msg.tile([128, np_ * 272], BF16, tag="msg")
            # leaky-relu for the pair (w folded in below; w >= 0)
            nc.scalar.activation(
                _ap(u_p, 0, [[16, np_], [8, 2], [1, 8]]),
                _ap(pft, 128, [[512, np_], [136, 2], [1, 8]]),
                mybir.ActivationFunctionType.Prelu,
                alpha=NEG_SLOPE,
            )
            # u *= w  (broadcast w over the 16 (b,h) slots)
            nc.gpsimd.tensor_tensor(
                u_p,
                u_p,
                _ap(wcol_sb, cbase[g] + c0, [[1, np_], [0, 16]]),
                MULT,
            )
            # ex = exp(u) -> msg_p[i*272 + 256 + b*8 + h]
            nc.scalar.activation(
                _ap(msg_p, 256, [[272, np_], [8, 2], [1, 8]]),
                _ap(u_p, 0, [[16, np_], [8, 2], [1, 8]]),
                mybir.ActivationFunctionType.Exp,
            )
            # msg = feat * ex (ex broadcast over dh via stride-0; feat columns
            # are dh-major so the broadcast repeats the contiguous 8-head run)
            for i in range(np_):
                nc.vector.tensor_tensor(
                    _ap(msg_p, i * 272, [[128, 2], [8, 16], [1, 8]]),
                    _ap(pft, i * 512, [[136, 2], [8, 16], [1, 8]]),
                    _ap(msg_p, i * 272 + 256, [[8, 2], [0, 16], [1, 8]]),
                    MULT,
                )

            def agg_closure():
                ncg_ = NC[g]
                if c0 == 0:
                    agg_t = pp_agg.tile([128, 512], F32, tag="agg")
                    aggs[g] = agg_t
                agg_ps = aggs[g]
                for i in range(np_):
                    c = c0 + i
                    nc.tensor.matmul(
                        agg_ps[:, 0:272], sts[i],
                        msg_p[:, i * 272 : (i + 1) * 272],
                        start=(c == 0), stop=(c == ncg_ - 1),
                    )

            return agg_closure

        def emit_epilogue(g):
            def closure():
                agg_ps = aggs.pop(g)
                sinv = p_fin.tile([128, 16], F32, tag="sinv")
                nc.vector.tensor_scalar_add(sinv, agg_ps[:, 256:272], 1e-30)
                nc.vector.reciprocal(sinv, sinv)
                aggn = p_fin.tile([128, 256], BF16, tag="aggn")
                # agg columns are (b, dh, h): sinv (b,h) broadcasts over dh
                nc.vector.tensor_tensor(
                    _ap(aggn, 0, [[128, 2], [8, 16], [1, 8]]),
                    _ap(agg_ps, 0, [[128, 2], [8, 16], [1, 8]]),
                    _ap(sinv, 0, [[8, 2], [0, 16], [1, 8]]),
                    MULT,
                )
                for b in range(2):
                    aggnT = p_fin.tile([128, 128], BF16, tag="aggnT")
                    nc.sync.dma_start_transpose(
                        aggnT, aggn[:, b * 128 : (b + 1) * 128]
                    )
                    psr = pp_epi.tile([128, 512], F32, tag="psr")
                    nc.tensor.matmul(psr, aggnT, wblk_sb, start=True, stop=True)
                    rst = p_fin.tile([128, 512], BF16, tag="rst")
                    if bias_nonzero:
                        nc.vector.tensor_tensor(rst, psr, bout_sb, ADD)
                    elif b == 0:
                        nc.scalar.activation(
                            rst, psr, mybir.ActivationFunctionType.Copy
                        )
                    else:
                        nc.vector.tensor_copy(rst, psr)
                    nc.sync.dma_start(out_d[b, g * 128 : (g + 1) * 128, :], rst)
            return closure

        pend = deque()
        for g in range(NG):
            for c0 in range(0, NC[g], 2):
                pend.append(emit_pair(g, c0))
                while len(pend) > LAG:
                    pend.popleft()()
            pend.append(emit_epilogue(g))
        while pend:
            pend.popleft()()

    nc.finalize()
    return nc


def _prep_host(x, src, dst, w, W_fc, b_fc, attn_l, attn_r, W_out, b_out):
    bf = ml_dtypes.bfloat16
    f8 = ml_dtypes.float8_e4m3
    x = np.asarray(x, np.float32)
    src = np.asarray(src).astype(np.int64)
    dst = np.asarray(dst).astype(np.int64)
    w = np.asarray(w, np.float32)
    W_fc = np.asarray(W_fc, np.float32)
    b_fc = np.asarray(b_fc, np.float32)
    al = np.asarray(attn_l, np.float32).reshape(H, DH)
    ar = np.asarray(attn_r, np.float32).reshape(H, DH)
    W_out = np.asarray(W_out, np.float32)
    b_out = np.asarray(b_out, np.float32)

    WfcT = np.ascontiguousarray(W_fc.T)                       # (d_in, d_out)
    W_el = np.einsum("dhk,hk->dh", WfcT.reshape(D, H, DH), al)
    # feat columns in dh-major order: col j*8+h = original dim h*16+j
    dperm = np.array([h * DH + j for j in range(DH) for h in range(H)])
    wcat = np.concatenate([WfcT[:, dperm], W_el], axis=1).astype(bf)  # (128,136)
    wer = np.einsum("dhk,hk->dh", WfcT.reshape(D, H, DH), ar).astype(bf)
    bel = np.einsum("hk,hk->h", b_fc.reshape(H, DH), al)
    ber = np.einsum("hk,hk->h", b_fc.reshape(H, DH), ar)
    cel = np.tile((bel + ber).astype(np.float32), (128, 1))   # (128, 8)
    wblk = np.zeros((D, 512), np.float32)
    for h in range(H):
        wblk[h * DH : (h + 1) * DH, h * OUT : (h + 1) * OUT] = W_out.T
    bfc_blk = b_fc @ wblk                                     # (512,)
    wblk = wblk[dperm, :]  # rows follow the dh-major feat layout
    bout_eff = np.tile(np.tile(b_out, H) + bfc_blk, (128, 1)).astype(np.float32)
    bias_nonzero = bool(np.abs(bout_eff).max() > 0)
    wblk = wblk.astype(bf)

    # ---- balanced graph partition: 160 (core, group) bins, <=128 nodes ----
    deg = np.bincount(dst, minlength=N)
    NB = NCORES * NG
    order_nodes = np.argsort(-deg, kind="stable")
    heap = [(0, 0, bid) for bid in range(NB)]  # (edge_total, node_cnt, bid)
    heapq.heapify(heap)
    node_bin = np.empty(N, np.int64)
    node_lid = np.empty(N, np.int64)
    for n in order_nodes:
        tot, cnt, bid = heapq.heappop(heap)
        node_bin[n] = bid
        node_lid[n] = cnt
        cnt += 1
        tot += int(deg[n])
        if cnt < 128:
            heapq.heappush(heap, (tot, cnt, bid))
    # bid = k * NG + g
    bin_k = node_bin // NG
    bin_g = node_bin % NG

    order = np.argsort(node_bin[dst], kind="stable")
    dsts, srcs, ws = dst[order], src[order], w[order]
    bins_sorted = node_bin[dsts]
    bounds = np.searchsorted(bins_sorted, np.arange(NB + 1))
    cnts = (bounds[1:] - bounds[:-1]).reshape(NCORES, NG)
    NC = np.maximum(1, ((cnts + 127) // 128).max(axis=0)).astype(int)  # (NG,)
    cbase = np.concatenate([[0], np.cumsum(NC)]).astype(int)
    TC = int(cbase[-1])

    xT = [np.ascontiguousarray(x[b].T.astype(bf)) for b in range(B)]  # (128, N)

    xeT = np.zeros((2, NCORES, 128, TC * 128), bf)
    S = np.zeros((NCORES, 128, TC * 128), f8)
    ST = np.zeros((NCORES, 128, TC * 128), f8)
    wcol = np.zeros((NCORES, 128, TC), np.float32)
    xgT = np.zeros((NCORES, 128, 2 * NPC), bf)
    # node -> (k, g, lid); own-dst columns for the er pass + out un-permute
    perm_node = np.full((NCORES, NPC), -1, np.int64)
    rows = bin_g * 128 + node_lid
    for k in range(NCORES):
        m = bin_k == np.int64(k)
        perm_node[k, rows[m]] = np.nonzero(m)[0]
        for b in range(B):
            xgT[k, :, b * NPC + rows[m]] = xT[b][:, m].T
    for bid in range(NB):
        k, g = bid // NG, bid % NG
        i0, i1 = bounds[bid], bounds[bid + 1]
        cnt = int(i1 - i0)
        if cnt == 0:
            continue
        j = np.arange(cnt)
        ch = cbase[g] + j // 128                           # global chunk
        sl = j % 128                                       # slot (edge row)
        lid = node_lid[dsts[i0:i1]]
        cols = ch * 128
        S[k, sl, cols + lid] = 1
        ST[k, lid, cols + sl] = 1
        wcol[k, sl, ch] = ws[i0:i1]
        for b in range(B):
            xeT[b, k, :, cols + sl] = xT[b][:, srcs[i0:i1]].T
    return (
        xeT, S, ST, wcol, xgT, wcat, wer, wblk, bout_eff, cel,
        list(map(int, NC)), bias_nonzero, perm_node,
    )


def kernel(vt=None, x=None, src=None, dst=None, w=None, W_fc=None, b_fc=None,
           attn_l=None, attn_r=None, W_out=None, b_out=None, **_ignored):
    global LAST_RESULTS
    (xeT, S, ST, wcol, xgT, wcat, wer, wblk, bout_eff, cel,
     NC, bias_nonzero, perm_node) = _prep_host(
        x, src, dst, w, W_fc, b_fc, attn_l, attn_r, W_out, b_out
    )
    nc = _build_program(NC, bias_nonzero)
    in_maps = []
    for k in range(NCORES):
        in_maps.append(
            dict(
                xeT0=np.ascontiguousarray(xeT[0, k]),
                xeT1=np.ascontiguousarray(xeT[1, k]),
                smat=np.ascontiguousarray(S[k]),
                stmat=np.ascontiguousarray(ST[k]),
                wcol=np.ascontiguousarray(wcol[k]),
                xgT=np.ascontiguousarray(xgT[k]),
                wcat=wcat,
                wer=wer,
                wblk=wblk,
                bout=bout_eff,
                cel=cel,
            )
        )
    res = run_bass_kernel_spmd(nc, in_maps, core_ids=list(range(NCORES)))
    LAST_RESULTS = res
    outs = [np.asarray(res.results[k]["out"], np.float32) for k in range(NCORES)]
    dev = np.concatenate(outs, axis=1)                   # (B, NCORES*NPC, 512)
    nodes = np.concatenate([perm_node[k] for k in range(NCORES)])
    valid = nodes >= 0
    full = np.empty((B, N, 512), np.float32)
    full[:, nodes[valid]] = dev[:, valid]
    return np.ascontiguousarray(full.reshape(B, N, H, OUT))


# revision 21
# speedup vs baseline: 3.6766x; 1.0157x over previous
"""GAT layer (nn_GATLayer) on 8 Trainium2 NeuronCores — gather-free design, v2.

Sharding: edges + output nodes sharded by dst node via balanced graph
partitioning (edge-cut per the hint); all FP compute on device; host does
integer graph partitioning and layout/weight reshuffling only (no FP math on
x beyond dtype conversion).

v2 changes vs v1 (282us):
  - PSUM pending-zero semantics exploited: start=True marks the whole 2KB
    bank pending-zero; later start=False writes to still-pending bytes
    OVERWRITE, writes to already-written bytes accumulate.  So per chunk
    only: fe0 (136c, start=T), fe1 (136c, start=F overwrite), er (16c
    strided-out accumulate), agg (272c).  The padded 272c claims / 136c er
    scatters / zcat zero matmul of v1 are gone: 816 -> 560 moving cols,
    7 -> 4 matmuls (= LDWEIGHTS) per chunk.
  - s-matmul folded into the agg matmul: the msg tile holds
    [msg_b0|msg_b1|ex_b0|ex_b1] (272 cols) so one matmul accumulates both
    agg and the softmax denominator.
  - w >= 0 so Lrelu(z*w) = w*Lrelu(z): per-chunk Prelu(scale=w) becomes a
    per-pair Prelu (no scale) + tiny gpsimd w-multiply + per-pair Exp.
    pf tiles are 2-bank PSUM tiles holding a chunk pair, so Prelu and the
    msg multiply batch across the pair.
  - ex broadcast over dh is materialized by an on-chip stride-0-read DMA,
    making the msg multiply a contiguous DVE op per pair.
  - agg matmuls trail the fe stream by LAG pairs (software pipelining) so
    the PE never stalls on the ACT->gpsimd->ACT->DMA->DVE msg chain.
  - Epilogue transpose via dma_start_transpose (XBAR): kills 40 PE
    transposes + 40 ACT copies + a PSUM bank.
  - S/ST shipped as fp8e4 (0/1 exact; matmul allows fp8 lhsT x bf16 rhs):
    halves their HBM traffic.  Output written bf16 (host casts to f32).
  - Host graph partition balances edges over the 160 (core, group) bins
    (each bin <= 128 dst nodes) so every group needs the same minimal
    chunk count (TC 350 -> 320); output rows un-permuted on host (integer
    gather only).
"""

import numpy as np
import ml_dtypes
import heapq
from collections import deque
from contextlib import ExitStack

import concourse.bass as bass
import concourse.bacc as bacc
import concourse.tile as tile
from concourse import mybir
from concourse.bass_utils import run_bass_kernel_spmd

B, N, D, H, DH, OUT = 2, 20000, 128, 8, 16, 64
E = 320000
NEG_SLOPE = 0.1
NCORES = 8
NG = 20                      # groups of <=128 dst nodes per core
NPC = NG * 128               # padded own-dst rows per core (2560)
F32 = mybir.dt.float32
BF16 = mybir.dt.bfloat16
FP8 = mybir.dt.float8e4
MULT = mybir.AluOpType.mult
ADD = mybir.AluOpType.add
SLAB = 16                    # chunks per DMA slab
TRI = 3                      # chunks per PSUM tile (3 banks)
LAG = 4                      # triples the agg matmuls trail behind the fe stream
LAG2 = 0                     # stage B (Exp+multiply) pops just before the next fe

LAST_RESULTS = None  # test harness can inspect exec_time_ns / profile


def _ap(t, off, dims):
    return bass.AP(tensor=t.tensor, offset=t.offset + off, ap=[t.ap[0]] + dims)


def _build_program(NC, bias_nonzero):
    TC = sum(NC)
    nc = bacc.Bacc(
        "TRN2", target_bir_lowering=False, debug=False, num_devices=NCORES
    )
    xeT0_d = nc.dram_tensor("xeT0", [128, TC * 128], BF16, kind="ExternalInput").ap()
    xeT1_d = nc.dram_tensor("xeT1", [128, TC * 128], BF16, kind="ExternalInput").ap()
    s_d = nc.dram_tensor("smat", [128, TC * 128], FP8, kind="ExternalInput").ap()
    st_d = nc.dram_tensor("stmat", [128, TC * 128], FP8, kind="ExternalInput").ap()
    wcol_d = nc.dram_tensor("wcol", [128, TC], F32, kind="ExternalInput").ap()
    xgT_d = nc.dram_tensor("xgT", [128, 2 * NPC], BF16, kind="ExternalInput").ap()
    wcat_d = nc.dram_tensor("wcat", [128, 136], BF16, kind="ExternalInput").ap()
    wer_d = nc.dram_tensor("wer", [128, 8], BF16, kind="ExternalInput").ap()
    wblk_d = nc.dram_tensor("wblk", [128, 512], BF16, kind="ExternalInput").ap()
    bout_d = nc.dram_tensor("bout", [128, 512], F32, kind="ExternalInput").ap()
    cel_d = nc.dram_tensor("cel", [128, 8], F32, kind="ExternalInput").ap()
    out_d = nc.dram_tensor("out", [B, NPC, 512], BF16, kind="ExternalOutput").ap()

    cbase = [0] + [int(v) for v in np.cumsum(NC)]

    with ExitStack() as ctx:
        tc = ctx.enter_context(tile.TileContext(nc))
        singles = ctx.enter_context(tc.tile_pool(name="singles", bufs=1))
        wcat_sb = singles.tile([128, 136], BF16)
        nc.sync.dma_start(wcat_sb, wcat_d)
        wer_sb = singles.tile([128, 8], BF16)
        nc.sync.dma_start(wer_sb, wer_d)
        wblk_sb = singles.tile([128, 512], BF16)
        nc.sync.dma_start(wblk_sb, wblk_d)
        bout_sb = singles.tile([128, 512], F32)
        nc.sync.dma_start(bout_sb, bout_d)
        cel_sb = singles.tile([128, 8], F32)
        nc.sync.dma_start(cel_sb, cel_d)
        wcol_sb = singles.tile([128, TC], F32)
        nc.sync.dma_start(wcol_sb, wcol_d)
        xg_sb = singles.tile([128, 2 * NPC], BF16)
        nc.sync.dma_start(xg_sb, xgT_d)
        # erp[m, g*16 + b*8 + h] = er of own-dst node (g, m) + (bel+ber)[h]
        erp = singles.tile([128, NG * 16], BF16)

        # ---- er pass ---------------------------------------------------------
        with ExitStack() as erctx:
            pp_er = erctx.enter_context(
                tc.tile_pool(name="pp_er", bufs=2, space="PSUM")
            )
            for b in range(2):
                for g in range(NG):
                    ps = pp_er.tile([128, 8], F32, tag="er")
                    nc.tensor.matmul(
                        ps, xg_sb[:, b * NPC + g * 128 : b * NPC + (g + 1) * 128],
                        wer_sb, start=True, stop=True,
                    )
                    nc.vector.tensor_tensor(
                        erp[:, g * 16 + b * 8 : g * 16 + b * 8 + 8],
                        ps, cel_sb, ADD,
                    )

        # ---- pools -----------------------------------------------------------
        xs0 = ctx.enter_context(tc.tile_pool(name="xs0", bufs=3))
        xs1 = ctx.enter_context(tc.tile_pool(name="xs1", bufs=3))
        ssl = ctx.enter_context(tc.tile_pool(name="ssl", bufs=3))
        stl = ctx.enter_context(tc.tile_pool(name="stl", bufs=3))
        p_u = ctx.enter_context(tc.tile_pool(name="p_u", bufs=LAG2 + 3))
        p_msg = ctx.enter_context(tc.tile_pool(name="p_msg", bufs=LAG + 4))
        p_fin = ctx.enter_context(tc.tile_pool(name="p_fin", bufs=6))
        pp_q = ctx.enter_context(tc.tile_pool(name="pp_q", bufs=2, space="PSUM"))
        pp_agg = ctx.enter_context(tc.tile_pool(name="pp_agg", bufs=1, space="PSUM"))
        pp_epi = ctx.enter_context(tc.tile_pool(name="pp_epi", bufs=1, space="PSUM"))

        slabs = {}

        def slab_tiles(cg):
            si = cg // SLAB
            if si not in slabs:
                n = min(SLAB, TC - si * SLAB) * 128
                o = si * SLAB * 128
                x0 = xs0.tile([128, n], BF16, tag="x0")
                nc.sync.dma_start(x0, xeT0_d[:, o : o + n])
                x1 = xs1.tile([128, n], BF16, tag="x1")
                nc.sync.dma_start(x1, xeT1_d[:, o : o + n])
                sm = ssl.tile([128, n], FP8, tag="sm")
                nc.sync.dma_start(sm, s_d[:, o : o + n])
                st = stl.tile([128, n], FP8, tag="st")
                nc.sync.dma_start(st, st_d[:, o : o + n])
                slabs[si] = (x0, x1, sm, st)
            k = (cg % SLAB) * 128
            x0, x1, sm, st = slabs[si]
            return (
                x0[:, k : k + 128], x1[:, k : k + 128],
                sm[:, k : k + 128], st[:, k : k + 128],
            )

        aggs = {}  # g -> agg_ps tile (allocated by the group's first agg)

        def emit_fe(g, c0):
            """Stage A: fe/er matmuls + Prelu + w-multiply for chunks
            [c0, c0+np_) of group g.  Returns (stageB, stageC) closures."""
            ncg = NC[g]
            np_ = min(TRI, ncg - c0)
            pft = pp_q.tile([128, TRI * 512], F32, tag="pf")
            sts = []
            for i in range(np_):
                cg = cbase[g] + c0 + i
                xe0, xe1, s_t, st_t = slab_tiles(cg)
                sts.append(s_t)
                pf = pft[:, i * 512 : i * 512 + 272]
                nc.tensor.matmul(pf[:, 0:136], xe0, wcat_sb,
                                 start=True, stop=False)
                nc.tensor.matmul(pf[:, 136:272], xe1, wcat_sb,
                                 start=False, stop=False)
                # el += er (strided out: cols 128:136 and 264:272)
                nc.tensor.matmul(
                    _ap(pf, 128, [[136, 2], [1, 8]]),
                    st_t, erp[:, g * 16 : g * 16 + 16],
                    start=False, stop=True,
                )
            u_p = p_u.tile([128, np_ * 16], F32, tag="u")
            msg_p = p_msg.tile([128, np_ * 272], BF16, tag="msg")
            # leaky-relu for the triple (w folded in below; w >= 0)
            nc.scalar.activation(
                _ap(u_p, 0, [[16, np_], [8, 2], [1, 8]]),
                _ap(pft, 128, [[512, np_], [136, 2], [1, 8]]),
                mybir.ActivationFunctionType.Prelu,
                alpha=NEG_SLOPE,
            )
            # u *= w  (broadcast w over the 16 (b,h) slots)
            nc.gpsimd.tensor_tensor(
                u_p,
                u_p,
                _ap(wcol_sb, cbase[g] + c0, [[1, np_], [0, 16]]),
                MULT,
            )

            def stage_b():
                # ex = exp(u) -> msg_p[i*272 + 256 + b*8 + h]
                nc.scalar.activation(
                    _ap(msg_p, 256, [[272, np_], [8, 2], [1, 8]]),
                    _ap(u_p, 0, [[16, np_], [8, 2], [1, 8]]),
                    mybir.ActivationFunctionType.Exp,
                )
                # msg = feat * ex (ex broadcast over dh via stride-0; feat
                # columns are dh-major so the broadcast repeats the
                # contiguous 8-head run); one op per batch b over the triple
                for b in range(2):
                    nc.vector.tensor_tensor(
                        _ap(msg_p, b * 128, [[272, np_], [8, 16], [1, 8]]),
                        _ap(pft, b * 136, [[512, np_], [8, 16], [1, 8]]),
                        _ap(msg_p, 256 + b * 8, [[272, np_], [0, 16], [1, 8]]),
                        MULT,
                    )

            def stage_c():
                ncg_ = NC[g]
                if c0 == 0:
                    agg_t = pp_agg.tile([128, 512], F32, tag="agg")
                    aggs[g] = agg_t
                agg_ps = aggs[g]
                for i in range(np_):
                    c = c0 + i
                    nc.tensor.matmul(
                        agg_ps[:, 0:272], sts[i],
                        msg_p[:, i * 272 : (i + 1) * 272],
                        start=(c == 0), stop=(c == ncg_ - 1),
                    )

            return stage_b, stage_c

        def emit_epilogue(g):
            def closure():
                agg_ps = aggs.pop(g)
                sinv = p_fin.tile([128, 16], F32, tag="sinv")
                nc.vector.tensor_scalar_add(sinv, agg_ps[:, 256:272], 1e-30)
                nc.vector.reciprocal(sinv, sinv)
                aggn = p_fin.tile([128, 256], BF16, tag="aggn")
                # agg columns are (b, dh, h): sinv (b,h) broadcasts over dh
                nc.vector.tensor_tensor(
                    _ap(aggn, 0, [[128, 2], [8, 16], [1, 8]]),
                    _ap(agg_ps, 0, [[128, 2], [8, 16], [1, 8]]),
                    _ap(sinv, 0, [[8, 2], [0, 16], [1, 8]]),
                    MULT,
                )
                for b in range(2):
                    aggnT = p_fin.tile([128, 128], BF16, tag="aggnT")
                    nc.sync.dma_start_transpose(
                        aggnT, aggn[:, b * 128 : (b + 1) * 128]
                    )
                    psr = pp_epi.tile([128, 512], F32, tag="psr")
                    nc.tensor.matmul(psr, aggnT, wblk_sb, start=True, stop=True)
                    rst = p_fin.tile([128, 512], BF16, tag="rst")
                    if bias_nonzero:
                        nc.vector.tensor_tensor(rst, psr, bout_sb, ADD)
                    else:
                        nc.scalar.activation(
                            rst, psr, mybir.ActivationFunctionType.Copy
                        )
                    nc.sync.dma_start(out_d[b, g * 128 : (g + 1) * 128, :], rst)
            return closure

        qb = deque()   # stage B (Exp + multiply), lag LAG2 triples
        qc = deque()   # stage C (agg matmuls) + epilogues, lag LAG triples
        for g in range(NG):
            for c0 in range(0, NC[g], TRI):
                while len(qb) > LAG2:
                    qb.popleft()()
                sb_, sc_ = emit_fe(g, c0)
                qb.append(sb_)
                qc.append(sc_)
                while len(qc) > LAG:
                    qc.popleft()()
            qc.append(emit_epilogue(g))
        while qb:
            qb.popleft()()
        while qc:
            qc.popleft()()

    nc.finalize()
    return nc


def _prep_host(x, src, dst, w, W_fc, b_fc, attn_l, attn_r, W_out, b_out):
    bf = ml_dtypes.bfloat16
    f8 = ml_dtypes.float8_e4m3
    x = np.asarray(x, np.float32)
    src = np.asarray(src).astype(np.int64)
    dst = np.asarray(dst).astype(np.int64)
    w = np.asarray(w, np.float32)
    W_fc = np.asarray(W_fc, np.float32)
    b_fc = np.asarray(b_fc, np.float32)
    al = np.asarray(attn_l, np.float32).reshape(H, DH)
    ar = np.asarray(attn_r, np.float32).reshape(H, DH)
    W_out = np.asarray(W_out, np.float32)
    b_out = np.asarray(b_out, np.float32)

    WfcT = np.ascontiguousarray(W_fc.T)                       # (d_in, d_out)
    W_el = np.einsum("dhk,hk->dh", WfcT.reshape(D, H, DH), al)
    # feat columns in dh-major order: col j*8+h = original dim h*16+j
    dperm = np.array([h * DH + j for j in range(DH) for h in range(H)])
    wcat = np.concatenate([WfcT[:, dperm], W_el], axis=1).astype(bf)  # (128,136)
    wer = np.einsum("dhk,hk->dh", WfcT.reshape(D, H, DH), ar).astype(bf)
    bel = np.einsum("hk,hk->h", b_fc.reshape(H, DH), al)
    ber = np.einsum("hk,hk->h", b_fc.reshape(H, DH), ar)
    cel = np.tile((bel + ber).astype(np.float32), (128, 1))   # (128, 8)
    wblk = np.zeros((D, 512), np.float32)
    for h in range(H):
        wblk[h * DH : (h + 1) * DH, h * OUT : (h + 1) * OUT] = W_out.T
    bfc_blk = b_fc @ wblk                                     # (512,)
    wblk = wblk[dperm, :]  # rows follow the dh-major feat layout
    bout_eff = np.tile(np.tile(b_out, H) + bfc_blk, (128, 1)).astype(np.float32)
    bias_nonzero = bool(np.abs(bout_eff).max() > 0)
    wblk = wblk.astype(bf)

    # ---- balanced graph partition: 160 (core, group) bins, <=128 nodes ----
    deg = np.bincount(dst, minlength=N)
    NB = NCORES * NG
    order_nodes = np.argsort(-deg, kind="stable")
    heap = [(0, 0, bid) for bid in range(NB)]  # (edge_total, node_cnt, bid)
    heapq.heapify(heap)
    node_bin = np.empty(N, np.int64)
    node_lid = np.empty(N, np.int64)
    for n in order_nodes:
        tot, cnt, bid = heapq.heappop(heap)
        node_bin[n] = bid
        node_lid[n] = cnt
        cnt += 1
        tot += int(deg[n])
        if cnt < 128:
            heapq.heappush(heap, (tot, cnt, bid))
    # bid = k * NG + g
    bin_k = node_bin // NG
    bin_g = node_bin % NG

    order = np.argsort(node_bin[dst], kind="stable")
    dsts, srcs, ws = dst[order], src[order], w[order]
    bins_sorted = node_bin[dsts]
    bounds = np.searchsorted(bins_sorted, np.arange(NB + 1))
    cnts = (bounds[1:] - bounds[:-1]).reshape(NCORES, NG)
    NC = np.maximum(1, ((cnts + 127) // 128).max(axis=0)).astype(int)  # (NG,)
    cbase = np.concatenate([[0], np.cumsum(NC)]).astype(int)
    TC = int(cbase[-1])

    xT = [np.ascontiguousarray(x[b].T.astype(bf)) for b in range(B)]  # (128, N)

    xeT = np.zeros((2, NCORES, 128, TC * 128), bf)
    S = np.zeros((NCORES, 128, TC * 128), f8)
    ST = np.zeros((NCORES, 128, TC * 128), f8)
    wcol = np.zeros((NCORES, 128, TC), np.float32)
    xgT = np.zeros((NCORES, 128, 2 * NPC), bf)
    # node -> (k, g, lid); own-dst columns for the er pass + out un-permute
    perm_node = np.full((NCORES, NPC), -1, np.int64)
    rows = bin_g * 128 + node_lid
    for k in range(NCORES):
        m = bin_k == np.int64(k)
        perm_node[k, rows[m]] = np.nonzero(m)[0]
        for b in range(B):
            xgT[k, :, b * NPC + rows[m]] = xT[b][:, m].T
    for bid in range(NB):
        k, g = bid // NG, bid % NG
        i0, i1 = bounds[bid], bounds[bid + 1]
        cnt = int(i1 - i0)
        if cnt == 0:
            continue
        j = np.arange(cnt)
        ch = cbase[g] + j // 128                           # global chunk
        sl = j % 128                                       # slot (edge row)
        lid = node_lid[dsts[i0:i1]]
        cols = ch * 128
        S[k, sl, cols + lid] = 1
        ST[k, lid, cols + sl] = 1
        wcol[k, sl, ch] = ws[i0:i1]
        for b in range(B):
            xeT[b, k, :, cols + sl] = xT[b][:, srcs[i0:i1]].T
    return (
        xeT, S, ST, wcol, xgT, wcat, wer, wblk, bout_eff, cel,
        list(map(int, NC)), bias_nonzero, perm_node,
    )


def kernel(vt=None, x=None, src=None, dst=None, w=None, W_fc=None, b_fc=None,
           attn_l=None, attn_r=None, W_out=None, b_out=None, **_ignored):
    global LAST_RESULTS
    (xeT, S, ST, wcol, xgT, wcat, wer, wblk, bout_eff, cel,
     NC, bias_nonzero, perm_node) = _prep_host(
        x, src, dst, w, W_fc, b_fc, attn_l, attn_r, W_out, b_out
    )
    nc = _build_program(NC, bias_nonzero)
    in_maps = []
    for k in range(NCORES):
        in_maps.append(
            dict(
                xeT0=np.ascontiguousarray(xeT[0, k]),
                xeT1=np.ascontiguousarray(xeT[1, k]),
                smat=np.ascontiguousarray(S[k]),
                stmat=np.ascontiguousarray(ST[k]),
                wcol=np.ascontiguousarray(wcol[k]),
                xgT=np.ascontiguousarray(xgT[k]),
                wcat=wcat,
                wer=wer,
                wblk=wblk,
                bout=bout_eff,
                cel=cel,
            )
        )
    res = run_bass_kernel_spmd(nc, in_maps, core_ids=list(range(NCORES)))
    LAST_RESULTS = res
    outs = [np.asarray(res.results[k]["out"], np.float32) for k in range(NCORES)]
    dev = np.concatenate(outs, axis=1)                   # (B, NCORES*NPC, 512)
    nodes = np.concatenate([perm_node[k] for k in range(NCORES)])
    valid = nodes >= 0
    full = np.empty((B, N, 512), np.float32)
    full[:, nodes[valid]] = dev[:, valid]
    return np.ascontiguousarray(full.reshape(B, N, H, OUT))
